# revision 1
# baseline (speedup 1.0000x reference)
"""Tensor-parallel GQA attention kernel for 8 Trainium2 NeuronCores.

Sharding: head-parallel. Core c computes q heads [4c, 4c+4) and kv head c
(GQA group). The output projection is row-sharded: each core multiplies its
local context features (512 of 4096) by its wo row-shard, producing a full
[512-seq, 4096] partial per seq tile, which a per-tile ReduceScatter sums
and shards by sequence rows. Host reassembles the 8 x 4 seq strips.

Attention processes query heads in pairs; the QKV projection, scores, PV,
and denominator matmuls all use fp16 operands (LDWEIGHTS for 16-bit
stationaries is ~2x faster, giving full 213ns/matmul cadence), while PSUM
accumulation stays fp32 and the output projection keeps wo in f32r. The
softmax denominator is accumulated exactly on the PE via an all-ones fp16
stationary into a broadcast PSUM tile.
"""

import math
import sys

import numpy as np

sys.path.insert(0, "/opt/trn_rl_repo")

# ---- problem constants (hardcoded per harness contract) ----
DIM = 4096
N_HEADS = 32
N_KV_HEADS = 8
HEAD_DIM = 128
N_REP = 4
SEQ = 2048
BATCH = 1
NCORES = 8

P = 128
KO = DIM // P        # 32 contraction chunks
SQ = 512             # seq tile width (matmul moving free dim)
NSQ = SEQ // SQ      # 4
NKS = SEQ // P       # 16 key tiles of 128
NH_LOC = N_HEADS // NCORES   # 4 local q heads
MQKV = NH_LOC * HEAD_DIM + 2 * HEAD_DIM  # 768 rows of fused qkv projection
SCALE = 1.0 / math.sqrt(HEAD_DIM)
OSH = SQ // NCORES   # 64 seq rows per core from each ReduceScatter

XB = 4               # k-chunks per xT load (1 MB DMAs)
JORDER = (1, 2, 3, 0)  # q-tile order: first phase3 waits least for the wo
                       # load; cheapest attention tile last shortens the tail

_CACHE = {}


def _build():
    """Build and compile the Bass kernel once per process."""
    if "nc" in _CACHE:
        return _CACHE["nc"]

    import concourse.bacc as bacc
    import concourse.mybir as mybir
    import concourse.tile as tile
    from concourse.masks import make_identity
    from contextlib import ExitStack

    F32 = mybir.dt.float32
    F32R = mybir.dt.float32r
    F16 = mybir.dt.float16
    MULT = mybir.AluOpType.mult
    ADD = mybir.AluOpType.add
    SUB = mybir.AluOpType.subtract
    EXP = mybir.ActivationFunctionType.Exp

    nc = bacc.Bacc(None, target_bir_lowering=False, debug=False)

    xT = nc.declare_dram_parameter("xt", [P, NSQ, KO, SQ], F16, isOutput=False)
    wqkv = nc.declare_dram_parameter("wqkv", [P, KO, MQKV], F16, isOutput=False)
    wo = nc.declare_dram_parameter("wo", [P, NH_LOC, DIM], F16, isOutput=False)
    cosd = nc.declare_dram_parameter("cost", [P, SEQ], F32, isOutput=False)
    sind = nc.declare_dram_parameter("sint", [P, SEQ], F32, isOutput=False)
    maskd = nc.declare_dram_parameter("masks", [P, 4, 2 * SQ], F16, isOutput=False)
    outs = [nc.declare_dram_parameter(f"o{j}", [OSH, DIM], F16, isOutput=True)
            for j in range(NSQ)]

    with tile.TileContext(nc) as tc, ExitStack() as stack:
        singles = stack.enter_context(tc.tile_pool(name="singles", bufs=1))
        dram = stack.enter_context(tc.tile_pool(name="dram", bufs=1, space="DRAM"))

        parts = [dram.tile([SQ, DIM], F16, name=f"part{j}") for j in range(NSQ)]
        rsouts = [dram.tile([OSH, DIM], F16, name=f"rsout{j}")
                  for j in range(NSQ)]

        idn = singles.tile([P, P], F32)
        make_identity(nc, idn)

        ones_f = singles.tile([P, P], F32)
        nc.vector.memset(ones_f[:], 1.0)
        expbias = singles.tile([P, 1], F32)
        nc.vector.memset(expbias[:], -7.0)
        ones128 = singles.tile([P, P], F16)
        nc.vector.tensor_copy(ones128[:], ones_f[:])

        # attention operands, resident across phases 1-2
        qsb = singles.tile([P, NH_LOC, SEQ], F16)   # per head: rows 0:64 re, 64:128 im
        kTsb = singles.tile([P, SEQ], F16)
        vsb = singles.tile([P, NKS, HEAD_DIM], F16)

        # ---------------- Phase 1: fused QKV projection + RoPE ----------------
        # m-tile order chosen so PSUM tiles are revisited in the order the
        # RoPE eviction frees them (pairs (0,3), (1,4), (2,5)).
        M_ORDER = (0, 3, 1, 4, 2, 5)
        with tc.tile_pool(name="wq", bufs=1) as wpool, \
             tc.tile_pool(name="xtp", bufs=2) as xpool, \
             tc.tile_pool(name="rt", bufs=2) as rpool, \
             tc.tile_pool(name="ps1", bufs=1, space="PSUM") as pp1:
            cos_sb = wpool.tile([P, SEQ], F32, tag="cos", name="cos_sb")
            sin_sb = wpool.tile([P, SEQ], F32, tag="sin", name="sin_sb")
            nc.sync.dma_start(cos_sb[:], cosd[:])
            nc.sync.dma_start(sin_sb[:], sind[:])
            vTsb = wpool.tile([P, SEQ], F32, tag="vT", name="vTsb")

            # weight tiles allocated up front; DMAs interleaved with the x
            # stream of the first sq tile so the first matmul starts after
            # ~2.6 MB instead of 13 MB
            w = [wpool.tile([P, 4, MQKV], F16, tag=f"w{g}", name=f"w{g}")
                 for g in range(KO // 4)]
            nc.sync.dma_start(w[0][:], wqkv[:, 0:4, :])

            def wslice(k, m):
                return w[k // 4][:, k % 4, m * P:(m + 1) * P]

            pending_tr = [None]
            for sq in range(NSQ):
                cols = slice(sq * SQ, (sq + 1) * SQ)
                pq = [pp1.tile([P, SQ], F32, tag=f"p{m}", name=f"p{m}_{sq}")
                      for m in range(6)]
                for xb in range(KO // XB):
                    xk = xpool.tile([P, XB, SQ], F16, tag="xt", name=f"x{sq}_{xb}")
                    nc.sync.dma_start(xk[:], xT[:, sq, xb * XB:(xb + 1) * XB, :])
                    if sq == 0 and xb + 1 < KO // 4:
                        nc.sync.dma_start(w[xb + 1][:],
                                          wqkv[:, 4 * (xb + 1):4 * (xb + 2), :])
                    for kk in range(XB):
                        k = xb * XB + kk
                        for m in M_ORDER:
                            nc.tensor.matmul(pq[m][:], wslice(k, m), xk[:, kk, :],
                                             start=(k == 0), stop=(k == KO - 1))
                    if xb == 0 and pending_tr[0] is not None:
                        pending_tr[0]()
                        pending_tr[0] = None

                # RoPE eviction. m-tile pairs: (0,3)->(q0,q1), (1,4)->(q2,q3),
                # (2,5)->(k | v-halves). Full-width multiplies first (frees the
                # PSUM pair after 4 ops), then 64-row combines into the heads.
                for i, (h0, h1) in enumerate(((0, 1), (2, 3), (4, 5))):
                    A, B = pq[i][:], pq[i + 3][:]
                    tac = rpool.tile([P, SQ], F32, tag="tac")   # A*cos
                    tas = rpool.tile([P, SQ], F32, tag="tas")   # A*sin
                    tbs = rpool.tile([P, SQ], F32, tag="tbs")   # B*sin
                    tbc = rpool.tile([P, SQ], F32, tag="tbc")   # B*cos
                    nc.vector.tensor_tensor(tac[:], A, cos_sb[:, cols], MULT)
                    nc.vector.tensor_tensor(tas[:], A, sin_sb[:, cols], MULT)
                    if i == 2:
                        # v passthrough straight from PSUM (frees pq[2]/pq[5])
                        nc.vector.tensor_copy(vTsb[0:64, cols], A[64:128])
                    nc.vector.tensor_tensor(tbs[:], B, sin_sb[:, cols], MULT)
                    nc.vector.tensor_tensor(tbc[:], B, cos_sb[:, cols], MULT)
                    if i == 2:
                        nc.vector.tensor_copy(vTsb[64:128, cols], B[64:128])
                        dests = ((slice(0, 64), kTsb[0:64, cols],
                                  kTsb[64:128, cols]),)
                    else:
                        h0q, h1q = 2 * i, 2 * i + 1
                        dests = ((slice(0, 64), qsb[0:64, h0q, cols],
                                  qsb[64:128, h0q, cols]),
                                 (slice(64, 128), qsb[0:64, h1q, cols],
                                  qsb[64:128, h1q, cols]))
                    for half, dre, dim_ in dests:
                        nc.vector.tensor_tensor(dre, tac[half], tbs[half], SUB)
                        nc.vector.tensor_tensor(dim_, tas[half], tbc[half], ADD)

                # transpose this quarter's v chunks: vT [128, s] -> v [s, 128]
                # (deferred into the next sq tile's matmul stream so the PE
                # doesn't stall here waiting for the RoPE vector ops)
                def mk_transposes(sq=sq):
                    def emit():
                        for t in range(4 * sq, 4 * sq + 4):
                            ptr = pp1.tile([P, P], F32, tag="ptr", bufs=2,
                                           name=f"ptr{t}")
                            nc.tensor.transpose(ptr[:],
                                                vTsb[:, t * P:(t + 1) * P],
                                                idn[:])
                            nc.scalar.copy(vsb[:, t, :], ptr[:])
                    return emit
                pending_tr[0] = mk_transposes()
                if sq == NSQ - 1:
                    pending_tr[0]()
                    pending_tr[0] = None

        # masks first (small, needed at the first attention tile), then wo
        mpool0 = stack.enter_context(tc.tile_pool(name="mp", bufs=1))
        mask_sb = mpool0.tile([P, 4, 2 * SQ], F16)
        nc.sync.dma_start(mask_sb[:], maskd[:])
        wopool = stack.enter_context(tc.tile_pool(name="wopool", bufs=1))
        wo_sb = wopool.tile([P, NH_LOC, DIM], F16)
        nc.sync.dma_start(wo_sb[:], wo[:])

        # ------- Phase 2+3: causal GQA attention + row-sharded out proj -------
        with tc.tile_pool(name="pt", bufs=3) as ptpool, \
             tc.tile_pool(name="st", bufs=2) as stpool, \
             tc.tile_pool(name="cx", bufs=2) as cxpool, \
             tc.tile_pool(name="os", bufs=4) as ospool, \
             tc.tile_pool(name="ps2", bufs=1, space="PSUM") as pp2:
            for j in JORDER:
                nks = 4 * (j + 1)
                qcols = slice(j * SQ, (j + 1) * SQ)
                ctx_sb = cxpool.tile([P, NH_LOC, SQ], F16, tag="cx",
                                     name=f"cx{j}")
                for hp in range(2):
                    h0, h1 = 2 * hp, 2 * hp + 1
                    ctx0 = pp2.tile([P, SQ], F32, tag="ctx", bufs=2,
                                    name=f"ctx{j}_{h0}")
                    ctx1 = pp2.tile([P, SQ], F32, tag="ctx", bufs=2,
                                    name=f"ctx{j}_{h1}")
                    # softmax denominator accumulated on the Vector engine
                    # (fp16 operands run at the 2x DVE rate); saves two PE
                    # matmuls per key tile
                    acc = stpool.tile([P, 2 * SQ], F16, tag="acc", bufs=2,
                                      name=f"acc{j}_{hp}")

                    # software pipeline: scores/exp run 2 tiles ahead of PV
                    def do_scores(t, j=j, qcols=qcols, h0=h0, h1=h1,
                                  acc=acc):
                        ps_s = pp2.tile([P, 2 * SQ], F32, tag="s", bufs=2,
                                        name=f"s{j}_{h0}_{t}")
                        kt = kTsb[:, t * P:(t + 1) * P]
                        nc.tensor.matmul(ps_s[:, 0:SQ], kt, qsb[:, h0, qcols],
                                         start=True, stop=True)
                        nc.tensor.matmul(ps_s[:, SQ:], kt, qsb[:, h1, qcols],
                                         start=True, stop=True)
                        pT = ptpool.tile([P, 2 * SQ], F16, tag="pT",
                                         name=f"pT{j}_{h0}_{t}")
                        # bias -7 keeps exp within fp16 range (max observed
                        # score*scale is ~11.5); numerator and denominator
                        # scale by the same e^-7, so softmax is unchanged
                        nc.scalar.activation(pT[:], ps_s[:], EXP, scale=SCALE,
                                             bias=expbias[:])
                        if t >= 4 * j:
                            nc.vector.tensor_tensor(pT[:], pT[:],
                                                    mask_sb[:, t - 4 * j, :],
                                                    MULT)
                        if t == 0:
                            nc.vector.tensor_copy(acc[:], pT[:])
                        else:
                            nc.vector.tensor_tensor(acc[:], acc[:], pT[:],
                                                    ADD)
                        return pT

                    def do_pv(t, pT, ctx0=ctx0, ctx1=ctx1, nks=nks):
                        vt = vsb[:, t, :]
                        nc.tensor.matmul(ctx0[:], vt, pT[:, 0:SQ],
                                         start=(t == 0), stop=(t == nks - 1))
                        nc.tensor.matmul(ctx1[:], vt, pT[:, SQ:],
                                         start=(t == 0), stop=(t == nks - 1))

                    pend = {}
                    for t in range(nks):
                        pend[t] = do_scores(t)
                        if t >= 2:
                            do_pv(t - 2, pend.pop(t - 2))
                    for t in (nks - 2, nks - 1):
                        do_pv(t, pend.pop(t))

                    # broadcast the denominator across partitions with an
                    # all-ones stationary, then normalize
                    bc = pp2.tile([P, 2 * SQ], F32, tag="bc", bufs=1,
                                  name=f"bc{j}_{hp}")
                    nc.tensor.matmul(bc[:, 0:SQ], ones128[:], acc[:, 0:SQ],
                                     start=True, stop=True)
                    nc.tensor.matmul(bc[:, SQ:], ones128[:], acc[:, SQ:],
                                     start=True, stop=True)
                    rc = stpool.tile([P, 2 * SQ], F32, tag="rc",
                                     name=f"rc{j}_{hp}")
                    nc.vector.reciprocal_approx_fast(rc[:], bc[:])
                    nc.vector.tensor_tensor(ctx_sb[:, h0, :], ctx0[:],
                                            rc[:, 0:SQ], MULT)
                    nc.vector.tensor_tensor(ctx_sb[:, h1, :], ctx1[:],
                                            rc[:, SQ:], MULT)

                # phase 3 for this seq tile: partial out = wo_rows^T @ ctx;
                # the ReduceScatter is split into two seq-halves so the
                # collective starts while the second half still computes
                for ssub in range(4):
                    srow = slice(ssub * P, (ssub + 1) * P)
                    for dp in range(4):
                        # po shares the (double-buffered) s-tag banks: the
                        # next group's matmuls overlap this group's eviction,
                        # keeping the PE continuously busy (p-state ramp)
                        po = pp2.tile([P, 2 * SQ], F32, tag="s", bufs=2,
                                      name=f"po{j}_{ssub}_{dp}")
                        for f in range(NH_LOC):
                            stat = ctx_sb[:, f, srow]
                            nc.tensor.matmul(
                                po[:, 0:SQ], stat,
                                wo_sb[:, f, dp * 2 * SQ:dp * 2 * SQ + SQ],
                                start=(f == 0), stop=(f == NH_LOC - 1))
                            nc.tensor.matmul(
                                po[:, SQ:], stat,
                                wo_sb[:, f, dp * 2 * SQ + SQ:(dp + 1) * 2 * SQ],
                                start=(f == 0), stop=(f == NH_LOC - 1))
                        osb = ospool.tile([P, 2 * SQ], F16, tag="osb",
                                          name=f"osb{j}_{ssub}_{dp}")
                        nc.vector.tensor_copy(osb[:, 0:SQ], po[:, 0:SQ])
                        nc.scalar.copy(osb[:, SQ:], po[:, SQ:])
                        nc.sync.dma_start(
                            parts[j][srow, dp * 2 * SQ:(dp + 1) * 2 * SQ],
                            osb[:])

                # one whole-tile ReduceScatter per seq tile: per-op overhead
                # (~30us) dwarfs the bandwidth term, so fewer/bigger ops win;
                # the input must be a whole tile — sliced collective inputs
                # lose their write-dependency tracking and race the DMAs
                nc.gpsimd.collective_compute(
                    "ReduceScatter", mybir.AluOpType.add,
                    replica_groups=[list(range(NCORES))],
                    ins=[parts[j][:]], outs=[rsouts[j][:]])


            # output copies go on the gpsimd queue: they wait on their
            # ReduceScatter, and the in-order SP DMA stream must never stall
            # behind a collective (it carries the partial-write DMAs)
            for j in JORDER:
                nc.gpsimd.dma_start(outs[j][:], rsouts[j][:])

    nc.compile()
    _CACHE["nc"] = nc
    return nc


def _prep_inputs(x, wq, wk, wv, wo, freqs_cos, freqs_sin):
    """Host-side sharding + layout prep. Returns in_maps for the 8 cores."""
    x = np.asarray(x, dtype=np.float32)
    wq = np.asarray(wq, dtype=np.float32)
    wk = np.asarray(wk, dtype=np.float32)
    wv = np.asarray(wv, dtype=np.float32)
    wo = np.asarray(wo, dtype=np.float32)
    freqs_cos = np.asarray(freqs_cos, dtype=np.float32)
    freqs_sin = np.asarray(freqs_sin, dtype=np.float32)

    # xT in [P, NSQ, KO, SQ] layout: element (d, s), d = ko*128 + p, s = sq*SQ + s'
    xT = np.ascontiguousarray(
        x[0].T.reshape(KO, P, NSQ, SQ).transpose(1, 2, 0, 3)).astype(np.float16)

    # rotate-half permutation within a head: [0,2,4,...126, 1,3,...,127]
    perm = np.concatenate([np.arange(0, HEAD_DIM, 2), np.arange(1, HEAD_DIM, 2)])

    # cos/sin tables transposed and duplicated across both 64-row halves
    cosT = np.ascontiguousarray(freqs_cos.T)  # [64, SEQ]
    sinT = np.ascontiguousarray(freqs_sin.T)
    cos2 = np.concatenate([cosT, cosT], axis=0)  # [128, SEQ]
    sin2 = np.concatenate([sinT, sinT], axis=0)

    # causal mask tiles: mask_r[i, jl] = 1 if jl - i >= 128*r, duplicated
    # across both halves of the head-pair score tile
    i_idx = np.arange(P)[:, None]
    j_idx = np.arange(SQ)[None, :]
    masks = np.stack([(j_idx - i_idx >= P * r).astype(np.float32)
                      for r in range(4)], axis=0)  # [4, 128, SQ]
    masks_l = np.ascontiguousarray(
        np.concatenate([masks, masks], axis=2).transpose(1, 0, 2)
    ).astype(np.float16)  # [P,4,2SQ]

    in_maps = []
    for c in range(NCORES):
        # fused qkv weight rows, permuted for RoPE (re/im separated by m-tile)
        qh = [wq[(4 * c + h) * HEAD_DIM:(4 * c + h + 1) * HEAD_DIM][perm]
              for h in range(NH_LOC)]  # each [128, DIM], rows [re(64); im(64)]
        kh = wk[c * HEAD_DIM:(c + 1) * HEAD_DIM][perm]  # [128, DIM]
        vh = wv[c * HEAD_DIM:(c + 1) * HEAD_DIM]        # [128, DIM] original order
        rows = np.empty((MQKV, DIM), dtype=np.float32)
        rows[0:64] = qh[0][0:64]        # tile0: q0 re | q1 re
        rows[64:128] = qh[1][0:64]
        rows[128:192] = qh[2][0:64]     # tile1: q2 re | q3 re
        rows[192:256] = qh[3][0:64]
        rows[256:320] = kh[0:64]        # tile2: k re | v dims 0:64
        rows[320:384] = vh[0:64]
        rows[384:448] = qh[0][64:128]   # tile3: q0 im | q1 im
        rows[448:512] = qh[1][64:128]
        rows[512:576] = qh[2][64:128]   # tile4: q2 im | q3 im
        rows[576:640] = qh[3][64:128]
        rows[640:704] = kh[64:128]      # tile5: k im | v dims 64:128
        rows[704:768] = vh[64:128]
        wqkvT = np.ascontiguousarray(
            rows.T.reshape(KO, P, MQKV).transpose(1, 0, 2)
        ).astype(np.float16)  # [P, KO, MQKV]

        # wo row shard, feature-major: woT[p, f, o] = wo[o, c*512 + f*128 + p]
        woT = np.ascontiguousarray(
            wo[:, c * NH_LOC * P:(c + 1) * NH_LOC * P].T
            .reshape(NH_LOC, P, DIM).transpose(1, 0, 2)).astype(np.float16)

        in_maps.append({
            "xt": xT,
            "wqkv": wqkvT,
            "wo": woT,
            "cost": cos2,
            "sint": sin2,
            "masks": masks_l,
        })
    return in_maps


def run(inputs, trace=False, tmpdir=None):
    """Compile (cached), run on 8 cores, return (output, BassKernelResults)."""
    from concourse.bass_utils import run_bass_kernel_spmd

    nc = _build()
    in_maps = _prep_inputs(**inputs)
    res = run_bass_kernel_spmd(nc, in_maps, list(range(NCORES)),
                               trace=trace, tmpdir=tmpdir)
    out = np.empty((BATCH, SEQ, DIM), dtype=np.float32)
    for c in range(NCORES):
        for j in range(NSQ):
            lo = j * SQ + c * OSH
            out[0, lo:lo + OSH, :] = np.asarray(res.results[c][f"o{j}"],
                                               dtype=np.float32)
    return out, res


def kernel(**inputs) -> np.ndarray:
    out, _ = run(inputs)
    return out



# revision 11
# speedup vs baseline: 1.0660x; 1.0660x over previous
"""Tensor-parallel GQA attention kernel for 8 Trainium2 NeuronCores.

Sharding: head-parallel. Core c computes q heads [4c, 4c+4) and kv head c
(GQA group). The output projection is row-sharded: each core multiplies its
local context features (512 of 4096) by its wo row-shard, producing a full
[512-seq, 4096] partial per seq tile, which a per-tile ReduceScatter sums
and shards by sequence rows. Host reassembles the 8 x 4 seq strips.

Attention processes query heads in pairs; the QKV projection, scores, PV,
and denominator matmuls all use fp16 operands (LDWEIGHTS for 16-bit
stationaries is ~2x faster, giving full 213ns/matmul cadence), while PSUM
accumulation stays fp32 and the output projection keeps wo in f32r. The
softmax denominator is accumulated exactly on the PE via an all-ones fp16
stationary into a broadcast PSUM tile.
"""

import math
import sys

import numpy as np

sys.path.insert(0, "/opt/trn_rl_repo")

# ---- problem constants (hardcoded per harness contract) ----
DIM = 4096
N_HEADS = 32
N_KV_HEADS = 8
HEAD_DIM = 128
N_REP = 4
SEQ = 2048
BATCH = 1
NCORES = 8

P = 128
KO = DIM // P        # 32 contraction chunks
SQ = 512             # seq tile width (matmul moving free dim)
NSQ = SEQ // SQ      # 4
NKS = SEQ // P       # 16 key tiles of 128
NH_LOC = N_HEADS // NCORES   # 4 local q heads
MQKV = NH_LOC * HEAD_DIM + 2 * HEAD_DIM  # 768 rows of fused qkv projection
SCALE = 1.0 / math.sqrt(HEAD_DIM)
OSH = SQ // NCORES   # 64 seq rows per core from each ReduceScatter

XB = 4               # k-chunks per xT load (1 MB DMAs)
JORDER = (1, 2, 3, 0)  # q-tile order: first phase3 waits least for the wo
                       # load; cheapest attention tile last shortens the tail

_CACHE = {}


def _build():
    """Build and compile the Bass kernel once per process."""
    if "nc" in _CACHE:
        return _CACHE["nc"]

    import concourse.bacc as bacc
    import concourse.mybir as mybir
    import concourse.tile as tile
    from concourse.masks import make_identity
    from contextlib import ExitStack

    F32 = mybir.dt.float32
    F32R = mybir.dt.float32r
    F16 = mybir.dt.float16
    MULT = mybir.AluOpType.mult
    ADD = mybir.AluOpType.add
    SUB = mybir.AluOpType.subtract
    EXP = mybir.ActivationFunctionType.Exp

    nc = bacc.Bacc(None, target_bir_lowering=False, debug=False)

    xT = nc.declare_dram_parameter("xt", [P, NSQ, KO, SQ], F16, isOutput=False)
    wqkv = nc.declare_dram_parameter("wqkv", [P, KO, MQKV], F16, isOutput=False)
    wo = nc.declare_dram_parameter("wo", [P, NH_LOC, DIM], F16, isOutput=False)
    cosd = nc.declare_dram_parameter("cost", [P, SEQ], F32, isOutput=False)
    sind = nc.declare_dram_parameter("sint", [P, SEQ], F32, isOutput=False)
    maskd = nc.declare_dram_parameter("masks", [P, 4, 2 * SQ], F16, isOutput=False)
    # the last-processed seq tile's output comes from two half-ReduceScatters
    outs = [nc.declare_dram_parameter(f"o{j}", [OSH, DIM], F16, isOutput=True)
            if j != JORDER[-1] else None for j in range(NSQ)]
    outs_h = [nc.declare_dram_parameter(f"o{JORDER[-1]}h{h}", [OSH // 2, DIM],
                                        F16, isOutput=True) for h in range(2)]

    with tile.TileContext(nc) as tc, ExitStack() as stack:
        singles = stack.enter_context(tc.tile_pool(name="singles", bufs=1))
        dram = stack.enter_context(tc.tile_pool(name="dram", bufs=1, space="DRAM"))

        # the last seq tile's partial is split in half so its ReduceScatter
        # can start while phase 3 still computes the second half
        JLAST = JORDER[-1]
        parts = [dram.tile([SQ, DIM], F16, name=f"part{j}") if j != JLAST
                 else None for j in range(NSQ)]
        parts_h = [dram.tile([SQ // 2, DIM], F16, name=f"part{JLAST}h{h}")
                   for h in range(2)]
        # collectives cannot write IO tensors — bounce through internal tiles
        rsouts = [dram.tile([OSH, DIM], F16, name=f"rsout{j}")
                  if j != JLAST else None for j in range(NSQ)]
        rsouts_h = [dram.tile([OSH // 2, DIM], F16, name=f"rsout{JLAST}h{h}")
                    for h in range(2)]

        idn = singles.tile([P, P], F32)
        make_identity(nc, idn)

        ones_f = singles.tile([P, P], F32)
        nc.vector.memset(ones_f[:], 1.0)
        expbias = singles.tile([P, 1], F32)
        nc.vector.memset(expbias[:], -7.0)
        ones128 = singles.tile([P, P], F16)
        nc.vector.tensor_copy(ones128[:], ones_f[:])

        # attention operands, resident across phases 1-2
        qsb = singles.tile([P, NH_LOC, SEQ], F16)   # per head: rows 0:64 re, 64:128 im
        kTsb = singles.tile([P, SEQ], F16)
        vsb = singles.tile([P, NKS, HEAD_DIM], F16)

        # ---------------- Phase 1: fused QKV projection + RoPE ----------------
        # m-tile order chosen so PSUM tiles are revisited in the order the
        # RoPE eviction frees them (pairs (0,3), (1,4), (2,5)).
        M_ORDER = (0, 3, 1, 4, 2, 5)
        with tc.tile_pool(name="wq", bufs=1) as wpool, \
             tc.tile_pool(name="xtp", bufs=2) as xpool, \
             tc.tile_pool(name="rt", bufs=2) as rpool, \
             tc.tile_pool(name="ps1", bufs=1, space="PSUM") as pp1:
            cos_sb = wpool.tile([P, SEQ], F32, tag="cos", name="cos_sb")
            sin_sb = wpool.tile([P, SEQ], F32, tag="sin", name="sin_sb")
            vTsb = wpool.tile([P, SEQ], F32, tag="vT", name="vTsb")

            # weight tiles allocated up front; DMAs interleaved with the x
            # stream of the first sq tile so the first matmul starts after
            # ~2.6 MB instead of 13 MB
            w = [wpool.tile([P, 4, MQKV], F16, tag=f"w{g}", name=f"w{g}")
                 for g in range(KO // 4)]
            nc.sync.dma_start(w[0][:], wqkv[:, 0:4, :])

            def wslice(k, m):
                return w[k // 4][:, k % 4, m * P:(m + 1) * P]

            pending_tr = [None]
            for sq in range(NSQ):
                cols = slice(sq * SQ, (sq + 1) * SQ)
                # allocate in eviction-pair order so the banks freed first by
                # the RoPE chain are the ones phase 2's first tiles land on
                pq = [None] * 6
                for m in (0, 3, 1, 4, 2, 5):
                    pq[m] = pp1.tile([P, SQ], F32, tag=f"p{m}", name=f"p{m}_{sq}")
                for xb in range(KO // XB):
                    xk = xpool.tile([P, XB, SQ], F16, tag="xt", name=f"x{sq}_{xb}")
                    nc.sync.dma_start(xk[:], xT[:, sq, xb * XB:(xb + 1) * XB, :])
                    if sq == 0 and xb + 1 < KO // 4:
                        nc.sync.dma_start(w[xb + 1][:],
                                          wqkv[:, 4 * (xb + 1):4 * (xb + 2), :])
                    if sq == 0 and xb == 0:
                        # cos/sin aren't needed until the first RoPE eviction
                        # (~45us in) — issue them behind the first x/w chunks
                        nc.sync.dma_start(cos_sb[:], cosd[:])
                        nc.sync.dma_start(sin_sb[:], sind[:])
                    for kk in range(XB):
                        k = xb * XB + kk
                        for m in M_ORDER:
                            nc.tensor.matmul(pq[m][:], wslice(k, m), xk[:, kk, :],
                                             start=(k == 0), stop=(k == KO - 1))
                    if xb == 0 and pending_tr[0] is not None:
                        pending_tr[0]()
                        pending_tr[0] = None

                # RoPE eviction. m-tile pairs: (0,3)->(q0,q1), (1,4)->(q2,q3),
                # (2,5)->(k | v-halves). Full-width multiplies first (frees the
                # PSUM pair after 4 ops), then 64-row combines into the heads.
                for i, (h0, h1) in enumerate(((0, 1), (2, 3), (4, 5))):
                    A, B = pq[i][:], pq[i + 3][:]
                    tac = rpool.tile([P, SQ], F32, tag="tac")   # A*cos
                    tas = rpool.tile([P, SQ], F32, tag="tas")   # A*sin
                    tbs = rpool.tile([P, SQ], F32, tag="tbs")   # B*sin
                    tbc = rpool.tile([P, SQ], F32, tag="tbc")   # B*cos
                    nc.vector.tensor_tensor(tac[:], A, cos_sb[:, cols], MULT)
                    nc.vector.tensor_tensor(tas[:], A, sin_sb[:, cols], MULT)
                    if i == 2:
                        # v passthrough straight from PSUM (frees pq[2]/pq[5])
                        nc.vector.tensor_copy(vTsb[0:64, cols], A[64:128])
                    nc.vector.tensor_tensor(tbs[:], B, sin_sb[:, cols], MULT)
                    nc.vector.tensor_tensor(tbc[:], B, cos_sb[:, cols], MULT)
                    if i == 2:
                        nc.vector.tensor_copy(vTsb[64:128, cols], B[64:128])
                        dests = ((slice(0, 64), kTsb[0:64, cols],
                                  kTsb[64:128, cols]),)
                    else:
                        h0q, h1q = 2 * i, 2 * i + 1
                        dests = ((slice(0, 64), qsb[0:64, h0q, cols],
                                  qsb[64:128, h0q, cols]),
                                 (slice(64, 128), qsb[0:64, h1q, cols],
                                  qsb[64:128, h1q, cols]))
                    for half, dre, dim_ in dests:
                        nc.vector.tensor_tensor(dre, tac[half], tbs[half], SUB)
                        nc.vector.tensor_tensor(dim_, tas[half], tbc[half], ADD)

                # transpose this quarter's v chunks: vT [128, s] -> v [s, 128]
                # (deferred into the next sq tile's matmul stream so the PE
                # doesn't stall here waiting for the RoPE vector ops)
                def mk_transposes(sq=sq):
                    def emit():
                        for t in range(4 * sq, 4 * sq + 4):
                            ptr = pp1.tile([P, P], F32, tag="ptr", bufs=2,
                                           name=f"ptr{t}")
                            nc.tensor.transpose(ptr[:],
                                                vTsb[:, t * P:(t + 1) * P],
                                                idn[:])
                            nc.scalar.copy(vsb[:, t, :], ptr[:])
                    return emit
                pending_tr[0] = mk_transposes()
                if sq == NSQ - 1:
                    pending_tr[0]()
                    pending_tr[0] = None

        # masks first (small, needed at the first attention tile), then wo
        mpool0 = stack.enter_context(tc.tile_pool(name="mp", bufs=1))
        mask_sb = mpool0.tile([P, 4, 2 * SQ], F16)
        nc.sync.dma_start(mask_sb[:], maskd[:])
        wopool = stack.enter_context(tc.tile_pool(name="wopool", bufs=1))
        wo_sb = wopool.tile([P, NH_LOC, DIM], F16)
        nc.sync.dma_start(wo_sb[:], wo[:])

        # ------- Phase 2+3: causal GQA attention + row-sharded out proj -------
        with tc.tile_pool(name="pt", bufs=3) as ptpool, \
             tc.tile_pool(name="st", bufs=2) as stpool, \
             tc.tile_pool(name="cx", bufs=2) as cxpool, \
             tc.tile_pool(name="os", bufs=10) as ospool, \
             tc.tile_pool(name="ps2", bufs=1, space="PSUM") as pp2:
            for j in JORDER:
                nks = 4 * (j + 1)
                ctx_sb = cxpool.tile([P, NH_LOC, SQ], F16, tag="cx",
                                     name=f"cx{j}")
                for hp in range(2):
                    h0, h1 = 2 * hp, 2 * hp + 1
                    ctx0 = pp2.tile([P, SQ], F32, tag="ctx", bufs=2,
                                    name=f"ctx{j}_{h0}")
                    ctx1 = pp2.tile([P, SQ], F32, tag="ctx", bufs=2,
                                    name=f"ctx{j}_{h1}")
                    # softmax denominator accumulated on the Vector engine
                    # (fp16 operands run at the 2x DVE rate); saves two PE
                    # matmuls per key tile
                    acc = stpool.tile([P, 2 * SQ], F16, tag="acc", bufs=2,
                                      name=f"acc{j}_{hp}")

                    # software pipeline: scores/exp run 2 tiles ahead of PV.
                    # Diagonal-block tiles (r = t-4j > 0) only attend queries
                    # q >= 128r, so scores/exp/mask/acc/PV are restricted to
                    # that column range (the excluded columns are exactly the
                    # fully-masked ones; acc/ctx keep their per-element
                    # accumulation correct because t=0 is always full-width)
                    def do_scores(t, j=j, h0=h0, h1=h1, acc=acc):
                        r = t - 4 * j if t >= 4 * j else 0
                        q0 = 128 * r
                        qc = slice(j * SQ + q0, (j + 1) * SQ)
                        ps_s = pp2.tile([P, 2 * SQ], F32, tag="s", bufs=2,
                                        name=f"s{j}_{h0}_{t}")
                        kt = kTsb[:, t * P:(t + 1) * P]
                        nc.tensor.matmul(ps_s[:, q0:SQ], kt, qsb[:, h0, qc],
                                         start=True, stop=True)
                        nc.tensor.matmul(ps_s[:, SQ + q0:], kt,
                                         qsb[:, h1, qc],
                                         start=True, stop=True)
                        pT = ptpool.tile([P, 2 * SQ], F16, tag="pT",
                                         name=f"pT{j}_{h0}_{t}")
                        # bias -7 keeps exp within fp16 range (max observed
                        # score*scale is ~11.5); numerator and denominator
                        # scale by the same e^-7, so softmax is unchanged
                        if q0 == 0:
                            nc.scalar.activation(pT[:], ps_s[:], EXP,
                                                 scale=SCALE, bias=expbias[:])
                        else:
                            nc.scalar.activation(pT[:, q0:SQ], ps_s[:, q0:SQ],
                                                 EXP, scale=SCALE,
                                                 bias=expbias[:])
                            nc.scalar.activation(pT[:, SQ + q0:],
                                                 ps_s[:, SQ + q0:], EXP,
                                                 scale=SCALE, bias=expbias[:])
                        if t >= 4 * j:
                            if q0 == 0:
                                nc.vector.tensor_tensor(
                                    pT[:], pT[:], mask_sb[:, r, :], MULT)
                            else:
                                nc.vector.tensor_tensor(
                                    pT[:, q0:SQ], pT[:, q0:SQ],
                                    mask_sb[:, r, q0:SQ], MULT)
                                nc.vector.tensor_tensor(
                                    pT[:, SQ + q0:], pT[:, SQ + q0:],
                                    mask_sb[:, r, SQ + q0:], MULT)
                        if t == 0:
                            nc.vector.tensor_copy(acc[:], pT[:])
                        elif q0 == 0:
                            nc.vector.tensor_tensor(acc[:], acc[:], pT[:],
                                                    ADD)
                        else:
                            nc.vector.tensor_tensor(acc[:, q0:SQ],
                                                    acc[:, q0:SQ],
                                                    pT[:, q0:SQ], ADD)
                            nc.vector.tensor_tensor(acc[:, SQ + q0:],
                                                    acc[:, SQ + q0:],
                                                    pT[:, SQ + q0:], ADD)
                        return pT

                    def do_pv(t, pT, ctx0=ctx0, ctx1=ctx1, nks=nks, j=j):
                        r = t - 4 * j if t >= 4 * j else 0
                        q0 = 128 * r
                        vt = vsb[:, t, :]
                        nc.tensor.matmul(ctx0[:, q0:], vt, pT[:, q0:SQ],
                                         start=(t == 0), stop=(t == nks - 1))
                        nc.tensor.matmul(ctx1[:, q0:], vt, pT[:, SQ + q0:],
                                         start=(t == 0), stop=(t == nks - 1))

                    pend = {}
                    for t in range(nks):
                        pend[t] = do_scores(t)
                        if t >= 2:
                            do_pv(t - 2, pend.pop(t - 2))
                    for t in (nks - 2, nks - 1):
                        do_pv(t, pend.pop(t))

                    # broadcast the denominator across partitions with an
                    # all-ones stationary, then normalize
                    bc = pp2.tile([P, 2 * SQ], F32, tag="bc", bufs=1,
                                  name=f"bc{j}_{hp}")
                    nc.tensor.matmul(bc[:, 0:SQ], ones128[:], acc[:, 0:SQ],
                                     start=True, stop=True)
                    nc.tensor.matmul(bc[:, SQ:], ones128[:], acc[:, SQ:],
                                     start=True, stop=True)
                    rc = stpool.tile([P, 2 * SQ], F32, tag="rc",
                                     name=f"rc{j}_{hp}")
                    nc.vector.reciprocal_approx_fast(rc[:], bc[:])
                    nc.vector.tensor_tensor(ctx_sb[:, h0, :], ctx0[:],
                                            rc[:, 0:SQ], MULT)
                    nc.vector.tensor_tensor(ctx_sb[:, h1, :], ctx1[:],
                                            rc[:, SQ:], MULT)

                # phase 3 for this seq tile: partial out = wo_rows^T @ ctx
                for ssub in range(4):
                    srow = slice(ssub * P, (ssub + 1) * P)
                    for dp in range(4):
                        # po shares the (double-buffered) s-tag banks: the
                        # next group's matmuls overlap this group's eviction,
                        # keeping the PE continuously busy (p-state ramp)
                        po = pp2.tile([P, 2 * SQ], F32, tag="s", bufs=2,
                                      name=f"po{j}_{ssub}_{dp}")
                        for f in range(NH_LOC):
                            stat = ctx_sb[:, f, srow]
                            nc.tensor.matmul(
                                po[:, 0:SQ], stat,
                                wo_sb[:, f, dp * 2 * SQ:dp * 2 * SQ + SQ],
                                start=(f == 0), stop=(f == NH_LOC - 1))
                            nc.tensor.matmul(
                                po[:, SQ:], stat,
                                wo_sb[:, f, dp * 2 * SQ + SQ:(dp + 1) * 2 * SQ],
                                start=(f == 0), stop=(f == NH_LOC - 1))
                        osb = ospool.tile([P, 2 * SQ], F16, tag="osb",
                                          name=f"osb{j}_{ssub}_{dp}")
                        nc.vector.tensor_copy(osb[:, 0:SQ], po[:, 0:SQ])
                        nc.scalar.copy(osb[:, SQ:], po[:, SQ:])
                        if j == JLAST:
                            nc.sync.dma_start(
                                parts_h[ssub // 2][
                                    srow.start - (ssub // 2) * 2 * P:
                                    srow.stop - (ssub // 2) * 2 * P,
                                    dp * 2 * SQ:(dp + 1) * 2 * SQ],
                                osb[:])
                        else:
                            nc.sync.dma_start(
                                parts[j][srow, dp * 2 * SQ:(dp + 1) * 2 * SQ],
                                osb[:])
                    # the last tile's ReduceScatter is split into seq-halves
                    # so the collective starts while phase 3 still computes
                    if j == JLAST and ssub == 1:
                        nc.gpsimd.collective_compute(
                            "ReduceScatter", mybir.AluOpType.add,
                            replica_groups=[list(range(NCORES))],
                            ins=[parts_h[0][:]], outs=[rsouts_h[0][:]])

                # one whole-tile ReduceScatter per seq tile otherwise: the
                # per-op floor dwarfs the bandwidth term, so fewer/bigger ops
                # win; inputs must be whole tiles — sliced collective inputs
                # lose their write-dependency tracking and race the DMAs.
                # Outputs go straight to the kernel output tensors.
                if j == JLAST:
                    nc.gpsimd.collective_compute(
                        "ReduceScatter", mybir.AluOpType.add,
                        replica_groups=[list(range(NCORES))],
                        ins=[parts_h[1][:]], outs=[rsouts_h[1][:]])
                else:
                    nc.gpsimd.collective_compute(
                        "ReduceScatter", mybir.AluOpType.add,
                        replica_groups=[list(range(NCORES))],
                        ins=[parts[j][:]], outs=[rsouts[j][:]])

            # output copies go on the gpsimd queue: each waits only on its
            # own ReduceScatter; all RS triggers were already enqueued above
            for j in JORDER:
                if j == JLAST:
                    nc.gpsimd.dma_start(outs_h[0][:], rsouts_h[0][:])
                    nc.gpsimd.dma_start(outs_h[1][:], rsouts_h[1][:])
                else:
                    nc.gpsimd.dma_start(outs[j][:], rsouts[j][:])

    nc.compile()
    _CACHE["nc"] = nc
    return nc


def _prep_inputs(x, wq, wk, wv, wo, freqs_cos, freqs_sin):
    """Host-side sharding + layout prep. Returns in_maps for the 8 cores."""
    x = np.asarray(x, dtype=np.float32)
    wq = np.asarray(wq, dtype=np.float32)
    wk = np.asarray(wk, dtype=np.float32)
    wv = np.asarray(wv, dtype=np.float32)
    wo = np.asarray(wo, dtype=np.float32)
    freqs_cos = np.asarray(freqs_cos, dtype=np.float32)
    freqs_sin = np.asarray(freqs_sin, dtype=np.float32)

    # xT in [P, NSQ, KO, SQ] layout: element (d, s), d = ko*128 + p, s = sq*SQ + s'
    xT = np.ascontiguousarray(
        x[0].T.reshape(KO, P, NSQ, SQ).transpose(1, 2, 0, 3)).astype(np.float16)

    # rotate-half permutation within a head: [0,2,4,...126, 1,3,...,127]
    perm = np.concatenate([np.arange(0, HEAD_DIM, 2), np.arange(1, HEAD_DIM, 2)])

    # cos/sin tables transposed and duplicated across both 64-row halves
    cosT = np.ascontiguousarray(freqs_cos.T)  # [64, SEQ]
    sinT = np.ascontiguousarray(freqs_sin.T)
    cos2 = np.concatenate([cosT, cosT], axis=0)  # [128, SEQ]
    sin2 = np.concatenate([sinT, sinT], axis=0)

    # causal mask tiles: mask_r[i, jl] = 1 if jl - i >= 128*r, duplicated
    # across both halves of the head-pair score tile
    i_idx = np.arange(P)[:, None]
    j_idx = np.arange(SQ)[None, :]
    masks = np.stack([(j_idx - i_idx >= P * r).astype(np.float32)
                      for r in range(4)], axis=0)  # [4, 128, SQ]
    masks_l = np.ascontiguousarray(
        np.concatenate([masks, masks], axis=2).transpose(1, 0, 2)
    ).astype(np.float16)  # [P,4,2SQ]

    in_maps = []
    for c in range(NCORES):
        # fused qkv weight rows, permuted for RoPE (re/im separated by m-tile)
        qh = [wq[(4 * c + h) * HEAD_DIM:(4 * c + h + 1) * HEAD_DIM][perm]
              for h in range(NH_LOC)]  # each [128, DIM], rows [re(64); im(64)]
        kh = wk[c * HEAD_DIM:(c + 1) * HEAD_DIM][perm]  # [128, DIM]
        vh = wv[c * HEAD_DIM:(c + 1) * HEAD_DIM]        # [128, DIM] original order
        rows = np.empty((MQKV, DIM), dtype=np.float32)
        rows[0:64] = qh[0][0:64]        # tile0: q0 re | q1 re
        rows[64:128] = qh[1][0:64]
        rows[128:192] = qh[2][0:64]     # tile1: q2 re | q3 re
        rows[192:256] = qh[3][0:64]
        rows[256:320] = kh[0:64]        # tile2: k re | v dims 0:64
        rows[320:384] = vh[0:64]
        rows[384:448] = qh[0][64:128]   # tile3: q0 im | q1 im
        rows[448:512] = qh[1][64:128]
        rows[512:576] = qh[2][64:128]   # tile4: q2 im | q3 im
        rows[576:640] = qh[3][64:128]
        rows[640:704] = kh[64:128]      # tile5: k im | v dims 64:128
        rows[704:768] = vh[64:128]
        wqkvT = np.ascontiguousarray(
            rows.T.reshape(KO, P, MQKV).transpose(1, 0, 2)
        ).astype(np.float16)  # [P, KO, MQKV]

        # wo row shard, feature-major: woT[p, f, o] = wo[o, c*512 + f*128 + p]
        woT = np.ascontiguousarray(
            wo[:, c * NH_LOC * P:(c + 1) * NH_LOC * P].T
            .reshape(NH_LOC, P, DIM).transpose(1, 0, 2)).astype(np.float16)

        in_maps.append({
            "xt": xT,
            "wqkv": wqkvT,
            "wo": woT,
            "cost": cos2,
            "sint": sin2,
            "masks": masks_l,
        })
    return in_maps


def run(inputs, trace=False, tmpdir=None):
    """Compile (cached), run on 8 cores, return (output, BassKernelResults)."""
    from concourse.bass_utils import run_bass_kernel_spmd

    nc = _build()
    in_maps = _prep_inputs(**inputs)
    res = run_bass_kernel_spmd(nc, in_maps, list(range(NCORES)),
                               trace=trace, tmpdir=tmpdir)
    out = np.empty((BATCH, SEQ, DIM), dtype=np.float32)
    jlast = JORDER[-1]
    for c in range(NCORES):
        for j in range(NSQ):
            if j == jlast:
                for h in range(2):
                    lo = j * SQ + h * (SQ // 2) + c * (OSH // 2)
                    out[0, lo:lo + OSH // 2, :] = np.asarray(
                        res.results[c][f"o{j}h{h}"], dtype=np.float32)
            else:
                lo = j * SQ + c * OSH
                out[0, lo:lo + OSH, :] = np.asarray(res.results[c][f"o{j}"],
                                                   dtype=np.float32)
    return out, res


def kernel(**inputs) -> np.ndarray:
    out, _ = run(inputs)
    return out

